# revision 6
# baseline (speedup 1.0000x reference)
"""Trainium2 Bass kernel for the AP-model RHS:
    out = concat(S @ u + 8*u*(1-u)*(u-par) - u*v,  -0.01*(8*u*(u-par-1) + v))
with D=8192, S row-sharded across 8 NeuronCores (1024 rows each).

Per-core dataflow (pure SPMD, no device collectives; the 8 KB row-concat
is done host-side):
  - stream the [1024, 8192] f32 row-shard of S in 8 tiles of [128, 8192]
  - one fused VectorE tensor_tensor_reduce per tile: multiply by the
    partition-broadcast u and row-reduce -> MK column (the matvec)
  - tiny elementwise reaction terms on [128, 8] tiles
The kernel is HBM-bound: 32 MB of S per core at ~360-420 GB/s.
"""

import numpy as np

import concourse.bacc as bacc
import concourse.mybir as mybir
import concourse.tile as tile
from concourse.bass_utils import run_bass_kernel_spmd

D = 8192
N_CORES = 8
ROWS = D // N_CORES          # 1024 rows of S per core
RT = ROWS // 128             # 8 row-tiles of 128 rows per core
F32 = mybir.dt.float32
K_PARAM = 8.0
EPS_PARAM = 0.01

_CACHE = {}


def build_nc():
    nc = bacc.Bacc("TRN2", target_bir_lowering=False, debug=False, num_devices=N_CORES)

    s_ext = nc.dram_tensor("s", [ROWS, D], F32, kind="ExternalInput")
    u_ext = nc.dram_tensor("u", [1, D], F32, kind="ExternalInput")
    # loc packs the per-core slices in [128, 8]-tile layout:
    # cols 0:RT = u_c, RT:2RT = v_c, 2RT:3RT = par_c  (loc[p, t] = x[t*128+p])
    loc_ext = nc.dram_tensor("loc", [128, 3 * RT], F32, kind="ExternalInput")
    out_ext = nc.dram_tensor("out", [128, 2 * RT], F32, kind="ExternalOutput")

    mult = mybir.AluOpType.mult
    add = mybir.AluOpType.add
    sub = mybir.AluOpType.subtract

    NCH = 4                      # chunks per chunked row-tile
    CH = D // NCH
    NCHUNKED = 2                 # the last NCHUNKED row-tiles stream in
    NBIG = RT - NCHUNKED         # [128, CH] chunks so the tail TTR is small
    with tile.TileContext(nc) as tc:
        with (
            tc.tile_pool(name="const", bufs=1) as const_pool,
            tc.tile_pool(name="s_pool", bufs=2) as s_pool,
            tc.tile_pool(name="ch_pool", bufs=NCH + 2) as ch_pool,
            tc.tile_pool(name="small", bufs=1) as small_pool,
        ):
            # u first on the sync queue (32 KB, so partition_broadcast can
            # run under the first S-tile DMA), then the big S-tile stream;
            # loc goes on the scalar engine's separate HWDGE queue.
            u_row = const_pool.tile([1, D], F32)
            nc.sync.dma_start(out=u_row[:], in_=u_ext[:])
            u_bcast = const_pool.tile([128, D], F32)
            nc.gpsimd.partition_broadcast(u_bcast[:], u_row[:])

            s_tiles = []
            for t in range(NBIG):
                s_tile = s_pool.tile([128, D], F32, tag="s_tile")
                nc.sync.dma_start(out=s_tile[:], in_=s_ext[t * 128:(t + 1) * 128, :])
                s_tiles.append(s_tile)
            ch_tiles = []
            for t in range(NBIG, RT):
                for j in range(NCH):
                    ch_tile = ch_pool.tile([128, CH], F32, tag="ch_tile")
                    nc.sync.dma_start(
                        out=ch_tile[:],
                        in_=s_ext[t * 128:(t + 1) * 128, j * CH:(j + 1) * CH],
                    )
                    ch_tiles.append(ch_tile)

            loc_sb = small_pool.tile([128, 3 * RT], F32)
            nc.scalar.dma_start(out=loc_sb[:], in_=loc_ext[:])

            # --- matvec: MK[t*128+p] = sum_k S[t*128+p, k] * u[k]
            # fused multiply + row-reduce on VectorE (native ISA op):
            # out = (s bypass) * u_bcast, accum_out = row-sum
            mk = small_pool.tile([128, RT], F32)
            for t in range(NBIG):
                nc.vector.scalar_tensor_tensor(
                    out=s_tiles[t][:],
                    in0=s_tiles[t][:],
                    scalar=1.0,
                    in1=u_bcast[:],
                    op0=mybir.AluOpType.bypass,
                    op1=mult,
                    accum_out=mk[:, t:t + 1],
                )
            mkc = small_pool.tile([128, NCHUNKED * NCH], F32)
            for i, ch_tile in enumerate(ch_tiles):
                j = i % NCH
                nc.vector.scalar_tensor_tensor(
                    out=ch_tile[:],
                    in0=ch_tile[:],
                    scalar=1.0,
                    in1=u_bcast[:, j * CH:(j + 1) * CH],
                    op0=mybir.AluOpType.bypass,
                    op1=mult,
                    accum_out=mkc[:, i:i + 1],
                )
            for t in range(NCHUNKED):
                nc.vector.tensor_reduce(
                    out=mk[:, NBIG + t:NBIG + t + 1],
                    in_=mkc[:, t * NCH:(t + 1) * NCH],
                    axis=mybir.AxisListType.X,
                    op=add,
                )

            # --- reaction terms on [128, RT] tiles (negligible work)
            u_t = loc_sb[:, 0:RT]
            v_t = loc_sb[:, RT:2 * RT]
            par_t = loc_sb[:, 2 * RT:3 * RT]

            out_sb = small_pool.tile([128, 2 * RT], F32)
            a = small_pool.tile([128, RT], F32)      # u*u
            b = small_pool.tile([128, RT], F32)      # u - u*u = u*(1-u)
            c = small_pool.tile([128, RT], F32)      # u - par
            d = small_pool.tile([128, RT], F32)      # u*(1-u)*(u-par)
            e = small_pool.tile([128, RT], F32)      # u*v
            f = small_pool.tile([128, RT], F32)      # 8*d - u*v
            g = small_pool.tile([128, RT], F32)      # u - par - 1
            h = small_pool.tile([128, RT], F32)      # u*(u-par-1)
            w = small_pool.tile([128, RT], F32)      # 0.01*v

            nc.vector.tensor_tensor(out=a[:], in0=u_t, in1=u_t, op=mult)
            nc.vector.tensor_tensor(out=b[:], in0=u_t, in1=a[:], op=sub)
            nc.vector.tensor_tensor(out=c[:], in0=u_t, in1=par_t, op=sub)
            nc.vector.tensor_tensor(out=d[:], in0=b[:], in1=c[:], op=mult)
            nc.vector.tensor_tensor(out=e[:], in0=u_t, in1=v_t, op=mult)
            # f = (d * 8) - e
            nc.vector.scalar_tensor_tensor(
                out=f[:], in0=d[:], scalar=K_PARAM, in1=e[:], op0=mult, op1=sub
            )
            # pde1 = mk + f
            nc.vector.tensor_tensor(out=out_sb[:, 0:RT], in0=mk[:, 0:RT], in1=f[:], op=add)
            # g = c - 1
            nc.vector.tensor_scalar_sub(out=g[:], in0=c[:], scalar1=1.0)
            nc.vector.tensor_tensor(out=h[:], in0=u_t, in1=g[:], op=mult)
            nc.vector.tensor_scalar_mul(out=w[:], in0=v_t, scalar1=EPS_PARAM)
            # pde2 = (h * -8*eps) - 0.01*v
            nc.vector.scalar_tensor_tensor(
                out=out_sb[:, RT:2 * RT], in0=h[:], scalar=-K_PARAM * EPS_PARAM,
                in1=w[:], op0=mult, op1=sub,
            )

            nc.sync.dma_start(out=out_ext[:], in_=out_sb[:])

    nc.compile()
    return nc


def _get_nc():
    if "nc" not in _CACHE:
        _CACHE["nc"] = build_nc()
    return _CACHE["nc"]


def _pack_col(x):
    # x: [1024] -> [128, 8] with loc[p, t] = x[t*128 + p]
    return np.ascontiguousarray(x.reshape(RT, 128).T)


def make_in_maps(y, S, par):
    u = y[:D]
    v = y[D:2 * D]
    par_flat = par.reshape(-1)
    u_row = np.ascontiguousarray(u.reshape(1, D))
    in_maps = []
    for c in range(N_CORES):
        sl = slice(c * ROWS, (c + 1) * ROWS)
        loc = np.empty((128, 3 * RT), np.float32)
        loc[:, 0:RT] = _pack_col(u[sl])
        loc[:, RT:2 * RT] = _pack_col(v[sl])
        loc[:, 2 * RT:3 * RT] = _pack_col(par_flat[sl])
        in_maps.append({
            "s": S[sl],
            "u": u_row,
            "loc": loc,
        })
    return in_maps


def assemble_output(results):
    full = np.empty(2 * D, np.float32)
    for c in range(N_CORES):
        o = results[c]["out"]            # [128, 16]
        full[c * ROWS:(c + 1) * ROWS] = o[:, 0:RT].T.reshape(-1)
        full[D + c * ROWS:D + (c + 1) * ROWS] = o[:, RT:2 * RT].T.reshape(-1)
    return full


def kernel(t=None, y=None, S=None, par=None, **_unused):
    y = np.asarray(y, np.float32)
    S = np.asarray(S, np.float32)
    par = np.asarray(par, np.float32)
    nc = _get_nc()
    in_maps = make_in_maps(y, S, par)
    res = run_bass_kernel_spmd(nc, in_maps, core_ids=list(range(N_CORES)))
    return assemble_output(res.results)


# revision 22
# speedup vs baseline: 478.5840x; 478.5840x over previous
"""Trainium2 Bass kernel for the AP-model RHS:
    out = concat(S @ u + 8*u*(1-u)*(u-par) - u*v,  -0.01*(8*u*(u-par-1) + v))
with D=8192, S row-sharded across 8 NeuronCores (1024 rows each).

Strategy (pure SPMD, no device collectives; the 8 KB row-concat happens
host-side):
  - the host hands each core a PACKED TRANSPOSE of its row-shard:
    st[p, jl*1024+m] = S[c*1024+m, (ti*J+jl)*128+p], so every DMA tile is
    a fully contiguous 4 MB block with the contraction dim on partitions
  - the matvec runs on the otherwise-idle TensorEngine as float32r
    matmuls (1 cycle/row): psum[1, m] += u_chunk[128,1].T @ st_tile[128, m]
    accumulated over all 64 k-chunks; the last 4 MB tile streams in 2 MB
    chunks so the tail is short; deep buffering (bufs=4) hides the
    per-tile DMA-completion/semaphore latency
  - reaction terms are a handful of [1, 1024] VectorE ops; pde1 adds the
    PSUM accumulator directly
The kernel is HBM-bound: 32 MB of S per core at the DMA streaming rate.
"""

import numpy as np

import concourse.bacc as bacc
import concourse.mybir as mybir
import concourse.tile as tile
from concourse.bass_utils import run_bass_kernel_spmd

D = 8192
N_CORES = 8
ROWS = D // N_CORES          # 1024 rows of S per core
NKC = D // 128               # 64 k-chunks of 128
F32 = mybir.dt.float32
F32R = mybir.dt.float32r
K_PARAM = 8.0
EPS_PARAM = 0.01

J = 8                        # k-chunks per big DMA tile (4 MB)
NBIGT = 7                    # 7 big tiles; the 8th streams as chunks
JC = 4                       # k-chunks per tail chunk (2 MB)
NCH = J // JC                # 2 tail chunks

_CACHE = {}


def _emit_body(nc, big_pool, ch_pool, small_pool, psum_pool,
               st_ext, loc_ext, out_ext, u_sb):
    mult = mybir.AluOpType.mult
    add = mybir.AluOpType.add
    sub = mybir.AluOpType.subtract

    acc = psum_pool.tile([1, ROWS], F32, tag="acc")

    loc_sb = small_pool.tile([1, 3 * ROWS], F32, tag="loc")
    nc.scalar.dma_start(out=loc_sb[:], in_=loc_ext[:])

    def matvec(tile_ap, jl_count, j0):
        # tile_ap: [128, jl_count*1024] slice of the packed transpose;
        # chunk jl holds k = (j0+jl)*128 + p
        for jl in range(jl_count):
            j = j0 + jl
            for h in range(2):
                nc.tensor.matmul(
                    acc[0:1, h * 512:(h + 1) * 512],
                    lhsT=u_sb[:, j:j + 1],
                    rhs=tile_ap[:, jl * ROWS + h * 512: jl * ROWS + (h + 1) * 512],
                    start=(j == 0),
                    stop=(j == NKC - 1),
                )

    for ti in range(NBIGT):
        s_tile = big_pool.tile([128, J * ROWS], F32R, tag="big")
        nc.sync.dma_start(
            out=s_tile[:], in_=st_ext[ti * 128:(ti + 1) * 128, :])
        matvec(s_tile[:], J, ti * J)
    for q in range(NCH):
        ch_tile = ch_pool.tile([128, JC * ROWS], F32R, tag="ch")
        nc.sync.dma_start(
            out=ch_tile[:],
            in_=st_ext[NBIGT * 128:(NBIGT + 1) * 128,
                       q * JC * ROWS:(q + 1) * JC * ROWS])
        matvec(ch_tile[:], JC, NBIGT * J + q * JC)

    # --- reaction terms on [1, 1024] tiles (DVE, overlapped w/ stream)
    u_t = loc_sb[0:1, 0:ROWS]
    v_t = loc_sb[0:1, ROWS:2 * ROWS]
    par_t = loc_sb[0:1, 2 * ROWS:3 * ROWS]
    out_sb = small_pool.tile([1, 2 * ROWS], F32, tag="osb")
    s1 = small_pool.tile([1, ROWS], F32, tag="s1")
    s2 = small_pool.tile([1, ROWS], F32, tag="s2")
    s3 = small_pool.tile([1, ROWS], F32, tag="s3")

    nc.vector.tensor_tensor(out=s1[:], in0=u_t, in1=par_t, op=sub)      # u-par
    nc.vector.tensor_scalar_sub(out=s2[:], in0=s1[:], scalar1=1.0)      # u-par-1
    nc.vector.tensor_tensor(out=s2[:], in0=u_t, in1=s2[:], op=mult)     # u(u-par-1)
    nc.vector.tensor_scalar_mul(out=s2[:], in0=s2[:],
                                scalar1=-K_PARAM * EPS_PARAM)
    nc.vector.tensor_scalar_mul(out=s3[:], in0=v_t, scalar1=EPS_PARAM)  # .01v
    nc.vector.tensor_tensor(out=out_sb[0:1, ROWS:2 * ROWS],
                            in0=s2[:], in1=s3[:], op=sub)               # pde2
    nc.vector.tensor_tensor(out=s2[:], in0=u_t, in1=u_t, op=mult)       # u^2
    nc.vector.tensor_tensor(out=s2[:], in0=u_t, in1=s2[:], op=sub)      # u(1-u)
    nc.vector.tensor_tensor(out=s2[:], in0=s2[:], in1=s1[:], op=mult)
    nc.vector.tensor_tensor(out=s3[:], in0=u_t, in1=v_t, op=mult)       # uv
    # s2 = 8*s2 - s3
    nc.vector.scalar_tensor_tensor(out=s2[:], in0=s2[:], scalar=K_PARAM,
                                   in1=s3[:], op0=mult, op1=sub)
    # pde1 = MK + s2  (reads the PSUM accumulator directly)
    nc.vector.tensor_tensor(out=out_sb[0:1, 0:ROWS], in0=acc[0:1, :],
                            in1=s2[:], op=add)

    nc.sync.dma_start(out=out_ext[:], in_=out_sb[:])


def build_nc(reps=1):
    nc = bacc.Bacc("TRN2", target_bir_lowering=False, debug=False,
                   num_devices=N_CORES)

    # packed transpose of the row-shard (see module docstring / make_in_maps)
    st_ext = nc.dram_tensor("st", [(NBIGT + 1) * 128, J * ROWS], F32R,
                            kind="ExternalInput")
    uc_ext = nc.dram_tensor("uc", [128, NKC], F32R, kind="ExternalInput")
    # loc = [u_c, v_c, par_c] on one row
    loc_ext = nc.dram_tensor("loc", [1, 3 * ROWS], F32, kind="ExternalInput")
    out_ext = nc.dram_tensor("out", [1, 2 * ROWS], F32, kind="ExternalOutput")

    with tile.TileContext(nc) as tc:
        with (
            tc.tile_pool(name="const", bufs=1) as const_pool,
            tc.tile_pool(name="big_pool", bufs=4) as big_pool,
            tc.tile_pool(name="ch_pool", bufs=2) as ch_pool,
            tc.tile_pool(name="small", bufs=1) as small_pool,
            tc.tile_pool(name="psum", bufs=4, space="PSUM") as psum_pool,
        ):
            u_sb = const_pool.tile([128, NKC], F32R)
            nc.sync.dma_start(out=u_sb[:], in_=uc_ext[:])
            for _rep in range(reps):
                _emit_body(nc, big_pool, ch_pool, small_pool, psum_pool,
                           st_ext, loc_ext, out_ext, u_sb)

    nc.compile()
    return nc


def _get_nc():
    if "nc" not in _CACHE:
        _CACHE["nc"] = build_nc()
    return _CACHE["nc"]


def make_in_maps(y, S, par):
    u = y[:D]
    v = y[D:2 * D]
    par_flat = par.reshape(-1)
    uc = np.ascontiguousarray(u.reshape(NKC, 128).T)
    in_maps = []
    for c in range(N_CORES):
        sl = slice(c * ROWS, (c + 1) * ROWS)
        # st[p, jl*1024+m] = S[c*1024+m, (ti*J+jl)*128+p]
        st = np.ascontiguousarray(
            S[sl].T.reshape(NBIGT + 1, J, 128, ROWS)
                   .transpose(0, 2, 1, 3)
                   .reshape((NBIGT + 1) * 128, J * ROWS))
        loc = np.concatenate([u[sl], v[sl], par_flat[sl]]).reshape(1, 3 * ROWS)
        in_maps.append({
            "st": st,
            "uc": uc,
            "loc": np.ascontiguousarray(loc),
        })
    return in_maps


def assemble_output(results):
    full = np.empty(2 * D, np.float32)
    for c in range(N_CORES):
        o = results[c]["out"][0]         # [2048]
        full[c * ROWS:(c + 1) * ROWS] = o[0:ROWS]
        full[D + c * ROWS:D + (c + 1) * ROWS] = o[ROWS:2 * ROWS]
    return full


def kernel(t=None, y=None, S=None, par=None, **_unused):
    y = np.asarray(y, np.float32)
    S = np.asarray(S, np.float32)
    par = np.asarray(par, np.float32)
    nc = _get_nc()
    in_maps = make_in_maps(y, S, par)
    res = run_bass_kernel_spmd(nc, in_maps, core_ids=list(range(N_CORES)))
    return assemble_output(res.results)


# revision 23
# speedup vs baseline: 483.7333x; 1.0108x over previous
"""Trainium2 Bass kernel for the AP-model RHS:
    out = concat(S @ u + 8*u*(1-u)*(u-par) - u*v,  -0.01*(8*u*(u-par-1) + v))
with D=8192, S row-sharded across 8 NeuronCores (1024 rows each).

Strategy (pure SPMD, no device collectives; the 8 KB row-concat happens
host-side):
  - the host hands each core a PACKED TRANSPOSE of its row-shard:
    st[p, jl*1024+m] = S[c*1024+m, (ti*J+jl)*128+p], so every DMA tile is
    a fully contiguous 4 MB block with the contraction dim on partitions
  - the matvec runs on the otherwise-idle TensorEngine as float32r
    matmuls (1 cycle/row): psum[1, m] += u_chunk[128,1].T @ st_tile[128, m]
    accumulated over all 64 k-chunks; the last 4 MB tile streams in 2 MB
    chunks (1 MB x4) so the tail is short; deep buffering (bufs=4) hides
    the per-tile DMA-completion/semaphore latency
  - reaction terms are a handful of [1, 1024] VectorE ops; pde1 adds the
    PSUM accumulator directly
The kernel is HBM-bound: 32 MB of S per core at the DMA streaming rate.
"""

import numpy as np

import concourse.bacc as bacc
import concourse.mybir as mybir
import concourse.tile as tile
from concourse.bass_utils import run_bass_kernel_spmd

D = 8192
N_CORES = 8
ROWS = D // N_CORES          # 1024 rows of S per core
NKC = D // 128               # 64 k-chunks of 128
F32 = mybir.dt.float32
F32R = mybir.dt.float32r
K_PARAM = 8.0
EPS_PARAM = 0.01

J = 8                        # k-chunks per big DMA tile (4 MB)
NBIGT = 7                    # 7 big tiles; the 8th streams as chunks
JC = 2                       # k-chunks per tail chunk (1 MB)
NCH = J // JC                # 2 tail chunks

_CACHE = {}


def _emit_body(nc, big_pool, ch_pool, small_pool, psum_pool,
               st_ext, loc_ext, out_ext, u_sb):
    mult = mybir.AluOpType.mult
    add = mybir.AluOpType.add
    sub = mybir.AluOpType.subtract

    acc = psum_pool.tile([1, ROWS], F32, tag="acc")

    loc_sb = small_pool.tile([1, 3 * ROWS], F32, tag="loc")
    nc.scalar.dma_start(out=loc_sb[:], in_=loc_ext[:])

    def matvec(tile_ap, jl_count, j0):
        # tile_ap: [128, jl_count*1024] slice of the packed transpose;
        # chunk jl holds k = (j0+jl)*128 + p
        for jl in range(jl_count):
            j = j0 + jl
            for h in range(2):
                nc.tensor.matmul(
                    acc[0:1, h * 512:(h + 1) * 512],
                    lhsT=u_sb[:, j:j + 1],
                    rhs=tile_ap[:, jl * ROWS + h * 512: jl * ROWS + (h + 1) * 512],
                    start=(j == 0),
                    stop=(j == NKC - 1),
                )

    for ti in range(NBIGT):
        s_tile = big_pool.tile([128, J * ROWS], F32R, tag="big")
        nc.sync.dma_start(
            out=s_tile[:], in_=st_ext[ti * 128:(ti + 1) * 128, :])
        matvec(s_tile[:], J, ti * J)
    for q in range(NCH):
        ch_tile = ch_pool.tile([128, JC * ROWS], F32R, tag="ch")
        nc.sync.dma_start(
            out=ch_tile[:],
            in_=st_ext[NBIGT * 128:(NBIGT + 1) * 128,
                       q * JC * ROWS:(q + 1) * JC * ROWS])
        matvec(ch_tile[:], JC, NBIGT * J + q * JC)

    # --- reaction terms on [1, 1024] tiles (DVE, overlapped w/ stream)
    u_t = loc_sb[0:1, 0:ROWS]
    v_t = loc_sb[0:1, ROWS:2 * ROWS]
    par_t = loc_sb[0:1, 2 * ROWS:3 * ROWS]
    out_sb = small_pool.tile([1, 2 * ROWS], F32, tag="osb")
    s1 = small_pool.tile([1, ROWS], F32, tag="s1")
    s2 = small_pool.tile([1, ROWS], F32, tag="s2")
    s3 = small_pool.tile([1, ROWS], F32, tag="s3")

    nc.vector.tensor_tensor(out=s1[:], in0=u_t, in1=par_t, op=sub)      # u-par
    nc.vector.tensor_scalar_sub(out=s2[:], in0=s1[:], scalar1=1.0)      # u-par-1
    nc.vector.tensor_tensor(out=s2[:], in0=u_t, in1=s2[:], op=mult)     # u(u-par-1)
    nc.vector.tensor_scalar_mul(out=s2[:], in0=s2[:],
                                scalar1=-K_PARAM * EPS_PARAM)
    nc.vector.tensor_scalar_mul(out=s3[:], in0=v_t, scalar1=EPS_PARAM)  # .01v
    nc.vector.tensor_tensor(out=out_sb[0:1, ROWS:2 * ROWS],
                            in0=s2[:], in1=s3[:], op=sub)               # pde2
    nc.vector.tensor_tensor(out=s2[:], in0=u_t, in1=u_t, op=mult)       # u^2
    nc.vector.tensor_tensor(out=s2[:], in0=u_t, in1=s2[:], op=sub)      # u(1-u)
    nc.vector.tensor_tensor(out=s2[:], in0=s2[:], in1=s1[:], op=mult)
    nc.vector.tensor_tensor(out=s3[:], in0=u_t, in1=v_t, op=mult)       # uv
    # s2 = 8*s2 - s3
    nc.vector.scalar_tensor_tensor(out=s2[:], in0=s2[:], scalar=K_PARAM,
                                   in1=s3[:], op0=mult, op1=sub)
    # pde1 = MK + s2  (reads the PSUM accumulator directly)
    nc.vector.tensor_tensor(out=out_sb[0:1, 0:ROWS], in0=acc[0:1, :],
                            in1=s2[:], op=add)

    nc.sync.dma_start(out=out_ext[:], in_=out_sb[:])


def build_nc(reps=1):
    nc = bacc.Bacc("TRN2", target_bir_lowering=False, debug=False,
                   num_devices=N_CORES)

    # packed transpose of the row-shard (see module docstring / make_in_maps)
    st_ext = nc.dram_tensor("st", [(NBIGT + 1) * 128, J * ROWS], F32R,
                            kind="ExternalInput")
    uc_ext = nc.dram_tensor("uc", [128, NKC], F32R, kind="ExternalInput")
    # loc = [u_c, v_c, par_c] on one row
    loc_ext = nc.dram_tensor("loc", [1, 3 * ROWS], F32, kind="ExternalInput")
    out_ext = nc.dram_tensor("out", [1, 2 * ROWS], F32, kind="ExternalOutput")

    with tile.TileContext(nc) as tc:
        with (
            tc.tile_pool(name="const", bufs=1) as const_pool,
            tc.tile_pool(name="big_pool", bufs=4) as big_pool,
            tc.tile_pool(name="ch_pool", bufs=4) as ch_pool,
            tc.tile_pool(name="small", bufs=1) as small_pool,
            tc.tile_pool(name="psum", bufs=4, space="PSUM") as psum_pool,
        ):
            u_sb = const_pool.tile([128, NKC], F32R)
            nc.sync.dma_start(out=u_sb[:], in_=uc_ext[:])
            for _rep in range(reps):
                _emit_body(nc, big_pool, ch_pool, small_pool, psum_pool,
                           st_ext, loc_ext, out_ext, u_sb)

    nc.compile()
    return nc


def _get_nc():
    if "nc" not in _CACHE:
        _CACHE["nc"] = build_nc()
    return _CACHE["nc"]


def make_in_maps(y, S, par):
    u = y[:D]
    v = y[D:2 * D]
    par_flat = par.reshape(-1)
    uc = np.ascontiguousarray(u.reshape(NKC, 128).T)
    in_maps = []
    for c in range(N_CORES):
        sl = slice(c * ROWS, (c + 1) * ROWS)
        # st[p, jl*1024+m] = S[c*1024+m, (ti*J+jl)*128+p]
        st = np.ascontiguousarray(
            S[sl].T.reshape(NBIGT + 1, J, 128, ROWS)
                   .transpose(0, 2, 1, 3)
                   .reshape((NBIGT + 1) * 128, J * ROWS))
        loc = np.concatenate([u[sl], v[sl], par_flat[sl]]).reshape(1, 3 * ROWS)
        in_maps.append({
            "st": st,
            "uc": uc,
            "loc": np.ascontiguousarray(loc),
        })
    return in_maps


def assemble_output(results):
    full = np.empty(2 * D, np.float32)
    for c in range(N_CORES):
        o = results[c]["out"][0]         # [2048]
        full[c * ROWS:(c + 1) * ROWS] = o[0:ROWS]
        full[D + c * ROWS:D + (c + 1) * ROWS] = o[ROWS:2 * ROWS]
    return full


def kernel(t=None, y=None, S=None, par=None, **_unused):
    y = np.asarray(y, np.float32)
    S = np.asarray(S, np.float32)
    par = np.asarray(par, np.float32)
    nc = _get_nc()
    in_maps = make_in_maps(y, S, par)
    res = run_bass_kernel_spmd(nc, in_maps, core_ids=list(range(N_CORES)))
    return assemble_output(res.results)
